# revision 11
# baseline (speedup 1.0000x reference)
"""Trainium2 Bass kernel for nn_BaseLineModel (segment_reduce).

Model: token-embed -> conv1d(K=3) -> relu -> global max-pool per note,
concat with delta-times, segment-mean over notes sharing a start day,
tiny linear + sigmoid -> [S, 1].

Sharding: notes (N=16384) split 8 ways, 2048 notes per core.

P1 (this rewrite): the embedding table lives in SBUF transposed
([128, V] f32: partitions 0:64 = emb.T, 64:128 = a second copy), and
gpsimd ap_gather produces the conv-ready slab directly: partition e of
slab col t = emb[tok_t, e] for e<64, and emb[tok_{t+1}, e-64] for
e>=64 (the k=1 shift is baked into per-16-partition index lists).
Conv = 2 PSUM-accumulated f32r matmul passes (contraction 128 for
k0|k1, 64 for k2 at col offset +2), DVE max-pool, ACT relu+bias.
P3: per-note z = feats . W[1:257] via matmuls; P4 segment-sum over
S=1024 days via one-hot matmuls; P5 ReduceScatter(add) + mean/linear/
sigmoid on each core's 128-day slice.
"""

import numpy as np

import concourse.bass as bass
import concourse.mybir as mybir
import concourse.tile as tile
from concourse.bass_utils import run_bass_kernel_spmd
from concourse import library_config

# ---- problem dims (hardcoded per task contract) ----
N, L, E, H, K, V, S = 16384, 64, 64, 256, 3, 30000, 1024
NCORES = 8
NC_NOTES = N // NCORES            # 2048 notes per core
NTOK = NC_NOTES * L               # 131072 tokens per core
BLK_NOTES = 128                   # notes per block
BLK_TOK = BLK_NOTES * L           # 8192 tokens per block
NBLK = NC_NOTES // BLK_NOTES      # 16
GCHUNK = 2048                     # tokens per ap_gather call (Q7 scratch cap)
NGC = BLK_TOK // GCHUNK           # 4 gather calls per block
NCHUNK = NC_NOTES // 128          # 16 note-chunks for segment phase
TMAX = L - K + 1                  # 62 valid conv positions

_SPLIT_MAXW = 1


def _split_waits(nc, maxw=_SPLIT_MAXW):
    """This walrus build rejects >1 sync wait per instruction; move extras
    onto preceding same-engine NOPs (sequencer order preserves semantics)."""
    for bb in nc.main_func.blocks:
        out = []
        for inst in bb.instructions:
            si = inst.sync_info
            waits = list(si.on_wait) if (si is not None and si.on_wait) else []
            if len(waits) > maxw:
                rest = waits[:-maxw]
                si.on_wait = waits[-maxw:]
                for i in range(0, len(rest), maxw):
                    out.append(mybir.InstNoOp(
                        name=f"{inst.name}-wsplit{i}",
                        sync_info=mybir.SyncInfo(on_wait=rest[i:i + maxw], on_update=[]),
                        bass_nofuse=True,
                        engine=inst.engine,
                    ))
            out.append(inst)
        bb.instructions = out


def _dedup_ldw(nc):
    """Drop PE InstLdweights that reload the stationary already resident
    (identical weights AP, no sync side effects). The PE array keeps the
    stationary across matmuls, so consecutive same-weight loads are no-ops."""
    for bb in nc.main_func.blocks:
        out = []
        prev = None
        for inst in bb.instructions:
            if type(inst).__name__ == 'InstLdweights':
                sig = str(inst.ins[0])
                si = inst.sync_info
                clean = not (si and (si.on_wait or si.on_update))
                if sig == prev and clean:
                    continue
                prev = sig
            out.append(inst)
        bb.instructions = out


def _build_nc(reps=1, use_cc=True, mode='full'):
    f32 = mybir.dt.float32
    f32r = mybir.dt.float32r  # noqa: F841
    bf16 = mybir.dt.bfloat16
    i16 = mybir.dt.int16

    nc = bass.Bass(num_swdge_queues=4)
    d_tab = nc.declare_dram_parameter("tab", [128, V], f32, isOutput=False)
    d_idx = nc.declare_dram_parameter("idx", [128, NTOK // 16], i16,
                                      isOutput=False)
    d_stf = nc.declare_dram_parameter("stf", [NC_NOTES, 1], f32, isOutput=False)
    d_w01 = nc.declare_dram_parameter("w01", [128, H], bf16, isOutput=False)
    d_w2 = nc.declare_dram_parameter("w2", [64, H], bf16, isOutput=False)
    d_cb = nc.declare_dram_parameter("convb2", [128, 2], f32, isOutput=False)
    d_iota = nc.declare_dram_parameter("iota", [128, S], f32, isOutput=False)
    d_wz = nc.declare_dram_parameter("wz", [128, 2], f32, isOutput=False)
    d_dw0 = nc.declare_dram_parameter("dw0", [1, NC_NOTES], f32, isOutput=False)
    d_brep = nc.declare_dram_parameter("brep", [128, 1], f32, isOutput=False)
    d_out = nc.declare_dram_parameter("out", [128, 1], f32, isOutput=True)
    part = nc.dram_tensor("part", [S, 2], f32)
    rs_out = nc.dram_tensor("rs_out", [128, 2], f32)

    with tile.TileContext(nc) as tc:
        nc.gpsimd.load_library(library_config.ap_gather)
        with (
            tc.tile_pool(name="cst", bufs=1) as cp,
            tc.tile_pool(name="feat", bufs=1) as fp,
        ):
         for _rep in range(reps):
             w01_sb = cp.tile([128, H], bf16, name="w01sb")
             w2_sb = cp.tile([64, H], bf16, name="w2sb")
             cb_sb = cp.tile([128, 2], f32, name="cbsb")
             nc.sync.dma_start(out=w01_sb[:], in_=d_w01[:])
             nc.sync.dma_start(out=w2_sb[:], in_=d_w2[:])
             nc.sync.dma_start(out=cb_sb[:], in_=d_cb[:])
             feats = [fp.tile([128, NC_NOTES], f32, name=f"feats{hh}") for hh in range(2)]

             # ---- P1: SBUF-resident table, ap_gather -> conv -> maxpool ----
             with tc.tile_pool(name="tabp", bufs=1) as tp:
                 tab = tp.tile([128, V], f32, name="tab")
                 nc.sync.dma_start(out=tab[:], in_=d_tab[:])
                 idx_all = tp.tile([128, NTOK // 16], i16, name="idxall")
                 nc.sync.dma_start(out=idx_all[:], in_=d_idx[:])
                 with (
                     tc.tile_pool(name="gath", bufs=1) as gp,
                     tc.tile_pool(name="ypsum", bufs=1, space="PSUM") as yp,
                 ):
                     for b in range(NBLK):
                         sbf = gp.tile([128, BLK_TOK], bf16, tag="sbf")
                         for hv in range(2):
                             slab = gp.tile([128, BLK_TOK // 2], f32, tag="slab")
                             for c in range(2):
                                 g = b * NGC + hv * 2 + c
                                 nc.gpsimd.ap_gather(
                                     out_ap=slab[:, c * GCHUNK:(c + 1) * GCHUNK]
                                         .rearrange("p (n d) -> p n d", d=1),
                                     in_ap=tab[:].rearrange("p (n d) -> p n d", d=1),
                                     idxs_ap=idx_all[:, g * (GCHUNK // 16):
                                                     (g + 1) * (GCHUNK // 16)],
                                     channels=128,
                                     num_elems=V,
                                     d=1,
                                     num_idxs=GCHUNK,
                                 )
                             nc.scalar.activation(
                                 out=sbf[:, hv * 4096:(hv + 1) * 4096],
                                 in_=slab[:],
                                 func=mybir.ActivationFunctionType.Copy)
                         if mode == 'gather':
                             continue
                         for hh in range(2):
                             for half in range(2):
                                 c0 = half * 4096
                                 y_ps = yp.tile([128, 4096], f32, tag="y",
                                                name=f"y{_rep}_{b}_{hh}_{half}")
                                 for o in range(0, 4096, 512):
                                     nc.tensor.matmul(
                                         out=y_ps[:, o:o + 512],
                                         lhsT=w01_sb[:, hh * 128:(hh + 1) * 128],
                                         rhs=sbf[:, c0 + o:c0 + o + 512],
                                         start=True, stop=False)
                                 for o in range(0, 4096, 512):
                                     w = 510 if (half == 1 and o == 3584) else 512
                                     nc.tensor.matmul(
                                         out=y_ps[:, o:o + w],
                                         lhsT=w2_sb[:, hh * 128:(hh + 1) * 128],
                                         rhs=sbf[0:64, c0 + o + 2:c0 + o + 2 + w],
                                         start=False, stop=True)
                                 nc.vector.reduce_max(
                                     out=feats[hh][:, b * BLK_NOTES + half * 64:
                                                   b * BLK_NOTES + half * 64 + 64],
                                     in_=y_ps[:].rearrange("p (n l) -> p n l", l=L)
                                         [:, :, 0:TMAX],
                                     axis=mybir.AxisListType.X)

             # ---- P2: relu(feats + conv_b) ----
             if mode == 'gather':
                 nc.vector.memset(feats[0][:], 0.0)
                 nc.vector.memset(feats[1][:], 0.0)
             for hh in range(2):
                 nc.scalar.activation(out=feats[hh][:], in_=feats[hh][:],
                                      func=mybir.ActivationFunctionType.Relu,
                                      bias=cb_sb[:, hh:hh + 1], scale=1.0)

             # ---- P3: per-note scalar z = feats . W[1:257] (+ W0*delta) ----
             with tc.tile_pool(name="zs", bufs=1) as zs:
                 wz_sb = zs.tile([128, 2], f32)
                 nc.sync.dma_start(out=wz_sb[:], in_=d_wz[:])
                 dw0_sb = zs.tile([1, NC_NOTES], f32)
                 nc.sync.dma_start(out=dw0_sb[:], in_=d_dw0[:])
                 z_sb = zs.tile([1, NC_NOTES], f32)
                 with tc.tile_pool(name="zp", bufs=2, space="PSUM") as zp:
                     for q in range(NC_NOTES // 512):
                         z_ps = zp.tile([1, 512], f32, tag="z", name=f"z{_rep}_{q}")
                         for hh in range(2):
                             nc.tensor.matmul(out=z_ps[:],
                                              lhsT=wz_sb[:, hh:hh + 1],
                                              rhs=feats[hh][:, q * 512:(q + 1) * 512],
                                              start=(hh == 0), stop=(hh == 1))
                         nc.vector.tensor_copy(out=z_sb[:, q * 512:(q + 1) * 512],
                                               in_=z_ps[:])
                 nc.vector.tensor_add(out=z_sb[:], in0=z_sb[:], in1=dw0_sb[:])

                 ones11 = zs.tile([1, 1], f32)
                 nc.vector.memset(ones11[:], 1.0)
                 rhs16 = zs.tile([128, 32], f32)
                 with tc.tile_pool(name="tpp", bufs=2, space="PSUM") as tpp:
                     for i in range(NCHUNK):
                         zt_ps = tpp.tile([128, 1], f32, tag="zt", name=f"zt{_rep}_{i}")
                         nc.tensor.matmul(out=zt_ps[:],
                                          lhsT=z_sb[0:1, i * 128:(i + 1) * 128],
                                          rhs=ones11[:],
                                          start=True, stop=True)
                         nc.vector.tensor_copy(out=rhs16[:, 2 * i:2 * i + 1],
                                               in_=zt_ps[:])
                     nc.vector.memset(rhs16[:, 1:32:2], 1.0)

                 # ---- P4: segment-sum of [z, 1] via one-hot matmuls ----
                 with (
                     tc.tile_pool(name="segsb", bufs=2) as ssp,
                     tc.tile_pool(name="segps", bufs=1, space="PSUM") as pp,
                 ):
                     iota_sb = zs.tile([128, S], f32)
                     nc.sync.dma_start(out=iota_sb[:], in_=d_iota[:])
                     seg_ps = [pp.tile([128, 2], f32, tag=f"seg{bk}",
                                       name=f"seg{_rep}_{bk}")
                               for bk in range(8)]
                     for i in range(NCHUNK):
                         st_sb = ssp.tile([128, 1], f32, tag="st")
                         nc.sync.dma_start(out=st_sb[:], in_=d_stf[i * 128:(i + 1) * 128, :])
                         oh_sb = ssp.tile([128, S], f32, tag="oh")
                         nc.vector.tensor_tensor(out=oh_sb[:],
                                                 in0=st_sb[:, 0:1].to_broadcast([128, S]),
                                                 in1=iota_sb[:],
                                                 op=mybir.AluOpType.is_equal)
                         for bk in range(8):
                             nc.tensor.matmul(out=seg_ps[bk][:],
                                              lhsT=oh_sb[:, bk * 128:(bk + 1) * 128],
                                              rhs=rhs16[:, 2 * i:2 * i + 2],
                                              start=(i == 0), stop=(i == NCHUNK - 1))
                     for bk in range(8):
                         seg_sb = ssp.tile([128, 2], f32, tag="segout")
                         nc.vector.tensor_copy(out=seg_sb[:], in_=seg_ps[bk][:])
                         nc.sync.dma_start(out=part[bk * 128:(bk + 1) * 128, :], in_=seg_sb[:])

             # ---- P5: cross-core reduce + finalize ----
             if use_cc:
                 with tc.tile_critical():
                     with nc.semaphore("cc_sem") as cc_sem:
                         nc.gpsimd.collective_compute(
                             "ReduceScatter", mybir.AluOpType.add,
                             replica_groups=[list(range(NCORES))],
                             ins=[part[:]], outs=[rs_out[:]],
                         ).then_inc(cc_sem, 1)
                         nc.gpsimd.wait_ge(cc_sem, 1)
             else:
                 nc.sync.dma_start(out=rs_out[:], in_=part[0:128, :])

             with tc.tile_pool(name="fin", bufs=1) as fin:
                 brep_sb = fin.tile([128, 1], f32)
                 nc.sync.dma_start(out=brep_sb[:], in_=d_brep[:])
                 fs = fin.tile([128, 2], f32)
                 nc.sync.dma_start(out=fs[:], in_=rs_out[:])
                 cnt = fin.tile([128, 1], f32)
                 nc.vector.tensor_scalar_max(out=cnt[:], in0=fs[:, 1:2], scalar1=1.0)
                 rcp = fin.tile([128, 1], f32)
                 nc.vector.reciprocal(out=rcp[:], in_=cnt[:])
                 dot = fin.tile([128, 1], f32)
                 nc.vector.tensor_tensor(out=dot[:], in0=fs[:, 0:1], in1=rcp[:],
                                         op=mybir.AluOpType.mult)
                 nc.vector.tensor_add(out=dot[:], in0=dot[:], in1=brep_sb[:])
                 outsb = fin.tile([128, 1], f32)
                 nc.scalar.activation(out=outsb[:], in_=dot[:],
                                      func=mybir.ActivationFunctionType.Sigmoid,
                                      scale=1.0)
                 nc.sync.dma_start(out=d_out[:], in_=outsb[:])

    _split_waits(nc)
    mybir.codegen_inst_isa_subclasses(nc)
    _dedup_ldw(nc)
    return nc


_NC_CACHE = {}


def _get_nc(reps=1, use_cc=True, mode='full'):
    key = (reps, use_cc, mode)
    if key not in _NC_CACHE:
        _NC_CACHE[key] = _build_nc(reps, use_cc, mode)
    return _NC_CACHE[key]


def _prep_inputs(text, start_times, emb, conv_w, conv_b, W, b):
    text = np.asarray(text)[0]              # [N, L]
    st = np.asarray(start_times)[0].astype(np.int64)   # [N]
    emb = np.asarray(emb, dtype=np.float32)
    conv_w = np.asarray(conv_w, dtype=np.float32)
    conv_b = np.asarray(conv_b, dtype=np.float32)
    W = np.asarray(W, dtype=np.float32)
    b = np.asarray(b, dtype=np.float32)

    # SBUF-resident transposed table: partitions 0:64 and 64:128 both emb.T
    tab = np.concatenate([emb.T, emb.T], axis=0).astype(np.float32)  # [128, V]
    tab = np.ascontiguousarray(tab)

    import ml_dtypes
    w01 = np.zeros((128, H), dtype=ml_dtypes.bfloat16)
    w01[:64, :] = conv_w[:, :, 0].T.astype(ml_dtypes.bfloat16)
    w01[64:, :] = conv_w[:, :, 1].T.astype(ml_dtypes.bfloat16)
    w2 = np.ascontiguousarray(conv_w[:, :, 2].T.astype(ml_dtypes.bfloat16))
    convb2 = np.ascontiguousarray(conv_b.reshape(2, 128).T.astype(np.float32))

    iota = np.tile(np.arange(S, dtype=np.float32), (128, 1))
    wz = np.ascontiguousarray(W[1:H + 1, 0].reshape(2, 128).T.astype(np.float32))
    brep = np.full((128, 1), b[0], np.float32)

    delta_g = np.concatenate([[0.0], np.diff(st).astype(np.float32)]).astype(np.float32)

    tok = text.astype(np.int16)             # V=30000 < 2**15
    in_maps = []
    for cid in range(NCORES):
        sl = slice(cid * NC_NOTES, (cid + 1) * NC_NOTES)
        flat = tok[sl].reshape(-1)                      # [NTOK] note-major
        # shifted-by-one stream for the k=1 conv tap (clamp last)
        flat_hi = np.concatenate([flat[1:], flat[:1]])
        ngc_all = NTOK // GCHUNK
        idx = np.zeros((128, NTOK // 16), np.int16)
        for g in range(ngc_all):
            lo = flat[g * GCHUNK:(g + 1) * GCHUNK]
            hi = flat_hi[g * GCHUNK:(g + 1) * GCHUNK]
            w_lo = lo.reshape(GCHUNK // 16, 16).T       # [16, GCHUNK//16]
            w_hi = hi.reshape(GCHUNK // 16, 16).T
            cs = slice(g * (GCHUNK // 16), (g + 1) * (GCHUNK // 16))
            idx[0:64, cs] = np.tile(w_lo, (4, 1))
            idx[64:128, cs] = np.tile(w_hi, (4, 1))
        in_maps.append({
            "tab": tab,
            "idx": idx,
            "stf": np.ascontiguousarray(st[sl, None].astype(np.float32)),
            "w01": w01,
            "w2": w2,
            "convb2": convb2,
            "iota": iota,
            "wz": wz,
            "dw0": np.ascontiguousarray((W[0, 0] * delta_g[sl])[None, :]),
            "brep": brep,
        })
    return in_maps


def kernel(**inputs) -> np.ndarray:
    nc = _get_nc()
    in_maps = _prep_inputs(**inputs)
    res = run_bass_kernel_spmd(nc, in_maps, list(range(NCORES))).results
    out = np.concatenate([res[c]["out"] for c in range(NCORES)], axis=0)
    return out.astype(np.float32)


if __name__ == "__main__":
    import jax
    import reference
    cpu = jax.devices("cpu")[0]
    with jax.default_device(cpu):
        ins = {k: np.asarray(v) for k, v in reference.setup_inputs().items()}
        exp = np.asarray(reference.reference(**reference.setup_inputs()))
    got = kernel(**ins)
    err = np.abs(got - exp).max()
    rel = err / max(np.abs(exp).max(), 1e-9)
    print("max abs err:", err, "rel:", rel)


# revision 13
# speedup vs baseline: 3.4629x; 3.4629x over previous
"""Trainium2 Bass kernel for nn_BaseLineModel (segment_reduce).

Model: token-embed -> conv1d(K=3) -> relu -> global max-pool per note,
concat with delta-times, segment-mean over notes sharing a start day,
tiny linear + sigmoid -> [S, 1].

Sharding: notes (N=16384) split 8 ways, 2048 notes per core.

P1 (this rewrite): the embedding table lives in SBUF transposed
([128, V] f32: partitions 0:64 = emb.T, 64:128 = a second copy), and
gpsimd ap_gather produces the conv-ready slab directly: partition e of
slab col t = emb[tok_t, e] for e<64, and emb[tok_{t+1}, e-64] for
e>=64 (the k=1 shift is baked into per-16-partition index lists).
Conv = 2 PSUM-accumulated f32r matmul passes (contraction 128 for
k0|k1, 64 for k2 at col offset +2), DVE max-pool, ACT relu+bias.
P3: per-note z = feats . W[1:257] via matmuls; P4 segment-sum over
S=1024 days via one-hot matmuls; P5 ReduceScatter(add) + mean/linear/
sigmoid on each core's 128-day slice.
"""

import numpy as np

import concourse.bass as bass
import concourse.mybir as mybir
import concourse.tile as tile
from concourse.bass_utils import run_bass_kernel_spmd
from concourse import library_config

# ---- problem dims (hardcoded per task contract) ----
N, L, E, H, K, V, S = 16384, 64, 64, 256, 3, 30000, 1024
NCORES = 8
NC_NOTES = N // NCORES            # 2048 notes per core
NTOK = NC_NOTES * L               # 131072 tokens per core
BLK_NOTES = 128                   # notes per block
BLK_TOK = BLK_NOTES * L           # 8192 tokens per block
NBLK = NC_NOTES // BLK_NOTES      # 16
GCHUNK = 2048                     # tokens per ap_gather call (Q7 scratch cap)
NGC = BLK_TOK // GCHUNK           # 4 gather calls per block
NCHUNK = NC_NOTES // 128          # 16 note-chunks for segment phase
TMAX = L - K + 1                  # 62 valid conv positions

_SPLIT_MAXW = 1


def _split_waits(nc, maxw=_SPLIT_MAXW):
    """This walrus build rejects >1 sync wait per instruction; move extras
    onto preceding same-engine NOPs (sequencer order preserves semantics)."""
    for bb in nc.main_func.blocks:
        out = []
        for inst in bb.instructions:
            si = inst.sync_info
            waits = list(si.on_wait) if (si is not None and si.on_wait) else []
            if len(waits) > maxw:
                rest = waits[:-maxw]
                si.on_wait = waits[-maxw:]
                for i in range(0, len(rest), maxw):
                    out.append(mybir.InstNoOp(
                        name=f"{inst.name}-wsplit{i}",
                        sync_info=mybir.SyncInfo(on_wait=rest[i:i + maxw], on_update=[]),
                        bass_nofuse=True,
                        engine=inst.engine,
                    ))
            out.append(inst)
        bb.instructions = out


def _dedup_ldw(nc):
    """Drop PE InstLdweights that reload the stationary already resident
    (identical weights AP, no sync side effects). The PE array keeps the
    stationary across matmuls, so consecutive same-weight loads are no-ops."""
    for bb in nc.main_func.blocks:
        out = []
        prev = None
        for inst in bb.instructions:
            if type(inst).__name__ == 'InstLdweights':
                sig = str(inst.ins[0])
                si = inst.sync_info
                clean = not (si and (si.on_wait or si.on_update))
                if sig == prev and clean:
                    continue
                prev = sig
            out.append(inst)
        bb.instructions = out


def _build_nc(reps=1, use_cc=True, mode='full'):
    f32 = mybir.dt.float32
    f32r = mybir.dt.float32r  # noqa: F841
    bf16 = mybir.dt.bfloat16
    i16 = mybir.dt.int16

    nc = bass.Bass(num_swdge_queues=4)
    d_tab = nc.declare_dram_parameter("tab", [128, V], f32, isOutput=False)
    d_idx = nc.declare_dram_parameter("idx", [128, NTOK // 16], i16,
                                      isOutput=False)
    d_stf = nc.declare_dram_parameter("stf", [NC_NOTES, 1], f32, isOutput=False)
    d_w01 = nc.declare_dram_parameter("w01", [128, H], bf16, isOutput=False)
    d_w2 = nc.declare_dram_parameter("w2", [64, H], bf16, isOutput=False)
    d_cb = nc.declare_dram_parameter("convb2", [128, 2], f32, isOutput=False)
    d_iota = nc.declare_dram_parameter("iota", [128, S], f32, isOutput=False)
    d_wz = nc.declare_dram_parameter("wz", [128, 2], f32, isOutput=False)
    d_dw0 = nc.declare_dram_parameter("dw0", [1, NC_NOTES], f32, isOutput=False)
    d_brep = nc.declare_dram_parameter("brep", [128, 1], f32, isOutput=False)
    d_out = nc.declare_dram_parameter("out", [128, 1], f32, isOutput=True)
    part = nc.dram_tensor("part", [S, 2], f32)
    rs_out = nc.dram_tensor("rs_out", [128, 2], f32)

    with tile.TileContext(nc) as tc:
        nc.gpsimd.load_library(library_config.ap_gather)
        with (
            tc.tile_pool(name="cst", bufs=1) as cp,
            tc.tile_pool(name="feat", bufs=1) as fp,
        ):
         for _rep in range(reps):
             w01_sb = cp.tile([128, H], bf16, name="w01sb")
             w2_sb = cp.tile([64, H], bf16, name="w2sb")
             cb_sb = cp.tile([128, 2], f32, name="cbsb")
             nc.sync.dma_start(out=w01_sb[:], in_=d_w01[:])
             nc.sync.dma_start(out=w2_sb[:], in_=d_w2[:])
             nc.sync.dma_start(out=cb_sb[:], in_=d_cb[:])
             feats = [fp.tile([128, NC_NOTES], f32, name=f"feats{hh}") for hh in range(2)]

             # ---- P1: SBUF-resident table, ap_gather -> conv -> maxpool ----
             with tc.tile_pool(name="tabp", bufs=1) as tp:
                 tab = tp.tile([128, V], f32, name="tab")
                 nc.sync.dma_start(out=tab[:], in_=d_tab[:])
                 idx_all = tp.tile([128, NTOK // 16], i16, name="idxall")
                 nc.sync.dma_start(out=idx_all[:], in_=d_idx[:])
                 with (
                     tc.tile_pool(name="gath", bufs=1) as gp,
                     tc.tile_pool(name="ypsum", bufs=1, space="PSUM") as yp,
                 ):
                     for b in range(NBLK):
                         sbf = gp.tile([128, BLK_TOK], bf16, tag="sbf")
                         for hv in range(2):
                             slab = gp.tile([128, BLK_TOK // 2], f32, tag="slab")
                             for c in range(2):
                                 g = b * NGC + hv * 2 + c
                                 nc.gpsimd.ap_gather(
                                     out_ap=slab[:, c * GCHUNK:(c + 1) * GCHUNK]
                                         .rearrange("p (n d) -> p n d", d=1),
                                     in_ap=tab[:].rearrange("p (n d) -> p n d", d=1),
                                     idxs_ap=idx_all[:, g * (GCHUNK // 16):
                                                     (g + 1) * (GCHUNK // 16)],
                                     channels=128,
                                     num_elems=V,
                                     d=1,
                                     num_idxs=GCHUNK,
                                 )
                             nc.scalar.activation(
                                 out=sbf[:, hv * 4096:(hv + 1) * 4096],
                                 in_=slab[:],
                                 func=mybir.ActivationFunctionType.Copy)
                         if mode == 'gather':
                             continue
                         for hh in range(2):
                             for half in range(2):
                                 c0 = half * 4096
                                 y_ps = yp.tile([128, 4096], f32, tag="y",
                                                name=f"y{_rep}_{b}_{hh}_{half}")
                                 for o in range(0, 4096, 512):
                                     nc.tensor.matmul(
                                         out=y_ps[:, o:o + 512],
                                         lhsT=w01_sb[:, hh * 128:(hh + 1) * 128],
                                         rhs=sbf[:, c0 + o:c0 + o + 512],
                                         start=True, stop=False)
                                 for o in range(0, 4096, 512):
                                     w = 510 if (half == 1 and o == 3584) else 512
                                     nc.tensor.matmul(
                                         out=y_ps[:, o:o + w],
                                         lhsT=w2_sb[:, hh * 128:(hh + 1) * 128],
                                         rhs=sbf[0:64, c0 + o + 2:c0 + o + 2 + w],
                                         start=False, stop=True)
                                 nc.vector.reduce_max(
                                     out=feats[hh][:, b * BLK_NOTES + half * 64:
                                                   b * BLK_NOTES + half * 64 + 64],
                                     in_=y_ps[:].rearrange("p (n l) -> p n l", l=L)
                                         [:, :, 0:TMAX],
                                     axis=mybir.AxisListType.X)

             # ---- P2: relu(feats + conv_b) ----
             if mode == 'gather':
                 nc.vector.memset(feats[0][:], 0.0)
                 nc.vector.memset(feats[1][:], 0.0)
             for hh in range(2):
                 nc.scalar.activation(out=feats[hh][:], in_=feats[hh][:],
                                      func=mybir.ActivationFunctionType.Relu,
                                      bias=cb_sb[:, hh:hh + 1], scale=1.0)

             # ---- P3: per-note scalar z = feats . W[1:257] (+ W0*delta) ----
             with tc.tile_pool(name="zs", bufs=1) as zs:
                 wz_sb = zs.tile([128, 2], f32)
                 nc.sync.dma_start(out=wz_sb[:], in_=d_wz[:])
                 dw0_sb = zs.tile([1, NC_NOTES], f32)
                 nc.sync.dma_start(out=dw0_sb[:], in_=d_dw0[:])
                 z_sb = zs.tile([1, NC_NOTES], f32)
                 with tc.tile_pool(name="zp", bufs=2, space="PSUM") as zp:
                     for q in range(NC_NOTES // 512):
                         z_ps = zp.tile([1, 512], f32, tag="z", name=f"z{_rep}_{q}")
                         for hh in range(2):
                             nc.tensor.matmul(out=z_ps[:],
                                              lhsT=wz_sb[:, hh:hh + 1],
                                              rhs=feats[hh][:, q * 512:(q + 1) * 512],
                                              start=(hh == 0), stop=(hh == 1))
                         nc.vector.tensor_copy(out=z_sb[:, q * 512:(q + 1) * 512],
                                               in_=z_ps[:])
                 nc.vector.tensor_add(out=z_sb[:], in0=z_sb[:], in1=dw0_sb[:])

                 ones11 = zs.tile([1, 1], f32)
                 nc.vector.memset(ones11[:], 1.0)
                 rhs16 = zs.tile([128, 32], f32)
                 with tc.tile_pool(name="tpp", bufs=2, space="PSUM") as tpp:
                     for i in range(NCHUNK):
                         zt_ps = tpp.tile([128, 1], f32, tag="zt", name=f"zt{_rep}_{i}")
                         nc.tensor.matmul(out=zt_ps[:],
                                          lhsT=z_sb[0:1, i * 128:(i + 1) * 128],
                                          rhs=ones11[:],
                                          start=True, stop=True)
                         nc.vector.tensor_copy(out=rhs16[:, 2 * i:2 * i + 1],
                                               in_=zt_ps[:])
                     nc.vector.memset(rhs16[:, 1:32:2], 1.0)

                 # ---- P4: segment-sum of [z, 1] via one-hot matmuls ----
                 with (
                     tc.tile_pool(name="segsb", bufs=2) as ssp,
                     tc.tile_pool(name="segps", bufs=1, space="PSUM") as pp,
                 ):
                     iota_sb = zs.tile([128, S], f32)
                     nc.sync.dma_start(out=iota_sb[:], in_=d_iota[:])
                     seg_ps = [pp.tile([128, 2], f32, tag=f"seg{bk}",
                                       name=f"seg{_rep}_{bk}")
                               for bk in range(8)]
                     for i in range(NCHUNK):
                         st_sb = ssp.tile([128, 1], f32, tag="st")
                         nc.sync.dma_start(out=st_sb[:], in_=d_stf[i * 128:(i + 1) * 128, :])
                         oh_sb = ssp.tile([128, S], f32, tag="oh")
                         nc.vector.tensor_tensor(out=oh_sb[:],
                                                 in0=st_sb[:, 0:1].to_broadcast([128, S]),
                                                 in1=iota_sb[:],
                                                 op=mybir.AluOpType.is_equal)
                         for bk in range(8):
                             nc.tensor.matmul(out=seg_ps[bk][:],
                                              lhsT=oh_sb[:, bk * 128:(bk + 1) * 128],
                                              rhs=rhs16[:, 2 * i:2 * i + 2],
                                              start=(i == 0), stop=(i == NCHUNK - 1))
                     for bk in range(8):
                         seg_sb = ssp.tile([128, 2], f32, tag="segout")
                         nc.vector.tensor_copy(out=seg_sb[:], in_=seg_ps[bk][:])
                         nc.sync.dma_start(out=part[bk * 128:(bk + 1) * 128, :], in_=seg_sb[:])

             # ---- P5: cross-core reduce + finalize ----
             if use_cc:
                 with tc.tile_critical():
                     with nc.semaphore("cc_sem") as cc_sem:
                         nc.gpsimd.collective_compute(
                             "ReduceScatter", mybir.AluOpType.add,
                             replica_groups=[list(range(NCORES))],
                             ins=[part[:]], outs=[rs_out[:]],
                         ).then_inc(cc_sem, 1)
                         nc.gpsimd.wait_ge(cc_sem, 1)
             else:
                 nc.sync.dma_start(out=rs_out[:], in_=part[0:128, :])

             with tc.tile_pool(name="fin", bufs=1) as fin:
                 brep_sb = fin.tile([128, 1], f32)
                 nc.sync.dma_start(out=brep_sb[:], in_=d_brep[:])
                 fs = fin.tile([128, 2], f32)
                 nc.sync.dma_start(out=fs[:], in_=rs_out[:])
                 cnt = fin.tile([128, 1], f32)
                 nc.vector.tensor_scalar_max(out=cnt[:], in0=fs[:, 1:2], scalar1=1.0)
                 rcp = fin.tile([128, 1], f32)
                 nc.vector.reciprocal(out=rcp[:], in_=cnt[:])
                 dot = fin.tile([128, 1], f32)
                 nc.vector.tensor_tensor(out=dot[:], in0=fs[:, 0:1], in1=rcp[:],
                                         op=mybir.AluOpType.mult)
                 nc.vector.tensor_add(out=dot[:], in0=dot[:], in1=brep_sb[:])
                 outsb = fin.tile([128, 1], f32)
                 nc.scalar.activation(out=outsb[:], in_=dot[:],
                                      func=mybir.ActivationFunctionType.Sigmoid,
                                      scale=1.0)
                 nc.sync.dma_start(out=d_out[:], in_=outsb[:])

    _split_waits(nc)
    mybir.codegen_inst_isa_subclasses(nc)
    _dedup_ldw(nc)
    return nc


_NC_CACHE = {}


def _get_nc(reps=1, use_cc=True, mode='full'):
    key = (reps, use_cc, mode)
    if key not in _NC_CACHE:
        _NC_CACHE[key] = _build_nc(reps, use_cc, mode)
    return _NC_CACHE[key]


def _prep_inputs(text, start_times, emb, conv_w, conv_b, W, b):
    text = np.asarray(text)[0]              # [N, L]
    st = np.asarray(start_times)[0].astype(np.int64)   # [N]
    emb = np.asarray(emb, dtype=np.float32)
    conv_w = np.asarray(conv_w, dtype=np.float32)
    conv_b = np.asarray(conv_b, dtype=np.float32)
    W = np.asarray(W, dtype=np.float32)
    b = np.asarray(b, dtype=np.float32)

    # SBUF-resident transposed table: partitions 0:64 and 64:128 both emb.T
    tab = np.concatenate([emb.T, emb.T], axis=0).astype(np.float32)  # [128, V]
    tab = np.ascontiguousarray(tab)

    import ml_dtypes
    w01 = np.zeros((128, H), dtype=ml_dtypes.bfloat16)
    w01[:64, :] = conv_w[:, :, 0].T.astype(ml_dtypes.bfloat16)
    w01[64:, :] = conv_w[:, :, 1].T.astype(ml_dtypes.bfloat16)
    w2 = np.ascontiguousarray(conv_w[:, :, 2].T.astype(ml_dtypes.bfloat16))
    convb2 = np.ascontiguousarray(conv_b.reshape(2, 128).T.astype(np.float32))

    iota = np.tile(np.arange(S, dtype=np.float32), (128, 1))
    wz = np.ascontiguousarray(W[1:H + 1, 0].reshape(2, 128).T.astype(np.float32))
    brep = np.full((128, 1), b[0], np.float32)

    delta_g = np.concatenate([[0.0], np.diff(st).astype(np.float32)]).astype(np.float32)

    tok = text.astype(np.int16)             # V=30000 < 2**15
    in_maps = []
    for cid in range(NCORES):
        sl = slice(cid * NC_NOTES, (cid + 1) * NC_NOTES)
        flat = tok[sl].reshape(-1)                      # [NTOK] note-major
        # shifted-by-one stream for the k=1 conv tap (clamp last)
        flat_hi = np.concatenate([flat[1:], flat[:1]])
        ngc_all = NTOK // GCHUNK
        idx = np.zeros((128, NTOK // 16), np.int16)
        for g in range(ngc_all):
            lo = flat[g * GCHUNK:(g + 1) * GCHUNK]
            hi = flat_hi[g * GCHUNK:(g + 1) * GCHUNK]
            w_lo = lo.reshape(GCHUNK // 16, 16).T       # [16, GCHUNK//16]
            w_hi = hi.reshape(GCHUNK // 16, 16).T
            cs = slice(g * (GCHUNK // 16), (g + 1) * (GCHUNK // 16))
            idx[0:64, cs] = np.tile(w_lo, (4, 1))
            idx[64:128, cs] = np.tile(w_hi, (4, 1))
        in_maps.append({
            "tab": tab,
            "idx": idx,
            "stf": np.ascontiguousarray(st[sl, None].astype(np.float32)),
            "w01": w01,
            "w2": w2,
            "convb2": convb2,
            "iota": iota,
            "wz": wz,
            "dw0": np.ascontiguousarray((W[0, 0] * delta_g[sl])[None, :]),
            "brep": brep,
        })
    return in_maps


def kernel(**inputs) -> np.ndarray:
    nc = _get_nc()
    in_maps = _prep_inputs(**inputs)
    res = run_bass_kernel_spmd(nc, in_maps, list(range(NCORES))).results
    out = np.concatenate([res[c]["out"] for c in range(NCORES)], axis=0)
    return out.astype(np.float32)


if __name__ == "__main__":
    import jax
    import reference
    cpu = jax.devices("cpu")[0]
    with jax.default_device(cpu):
        ins = {k: np.asarray(v) for k, v in reference.setup_inputs().items()}
        exp = np.asarray(reference.reference(**reference.setup_inputs()))
    got = kernel(**ins)
    err = np.abs(got - exp).max()
    rel = err / max(np.abs(exp).max(), 1e-9)
    print("max abs err:", err, "rel:", rel)


# revision 18
# speedup vs baseline: 5.7164x; 1.6508x over previous
"""Trainium2 Bass kernel for nn_BaseLineModel (segment_reduce).

Model: token-embed -> conv1d(K=3) -> relu -> global max-pool per note,
concat with delta-times, segment-mean over notes sharing a start day,
tiny linear + sigmoid -> [S, 1].

Sharding: notes (N=16384) split 8 ways, 2048 notes per core.

P1 (this rewrite): the embedding table lives in SBUF transposed
([128, V] f32: partitions 0:64 = emb.T, 64:128 = a second copy), and
gpsimd ap_gather produces the conv-ready slab directly: partition e of
slab col t = emb[tok_t, e] for e<64, and emb[tok_{t+1}, e-64] for
e>=64 (the k=1 shift is baked into per-16-partition index lists).
Conv = 2 PSUM-accumulated f32r matmul passes (contraction 128 for
k0|k1, 64 for k2 at col offset +2), DVE max-pool, ACT relu+bias.
P3: per-note z = feats . W[1:257] via matmuls; P4 segment-sum over
S=1024 days via one-hot matmuls; P5 ReduceScatter(add) + mean/linear/
sigmoid on each core's 128-day slice.
"""

import numpy as np

import concourse.bass as bass
import concourse.mybir as mybir
import concourse.tile as tile
from concourse.bass_utils import run_bass_kernel_spmd
from concourse import library_config

# ---- problem dims (hardcoded per task contract) ----
N, L, E, H, K, V, S = 16384, 64, 64, 256, 3, 30000, 1024
NCORES = 8
NC_NOTES = N // NCORES            # 2048 notes per core
NTOK = NC_NOTES * L               # 131072 tokens per core
BLK_NOTES = 128                   # notes per block
BLK_TOK = BLK_NOTES * L           # 8192 tokens per block
NBLK = NC_NOTES // BLK_NOTES      # 16
GCHUNK = 4096                     # tokens per ap_gather call
NGC = BLK_TOK // GCHUNK           # 2 gather calls per block
NCHUNK = NC_NOTES // 128          # 16 note-chunks for segment phase
TMAX = L - K + 1                  # 62 valid conv positions

_SPLIT_MAXW = 1


def _split_waits(nc, maxw=_SPLIT_MAXW):
    """This walrus build rejects >1 sync wait per instruction; move extras
    onto preceding same-engine NOPs (sequencer order preserves semantics)."""
    for bb in nc.main_func.blocks:
        out = []
        for inst in bb.instructions:
            si = inst.sync_info
            waits = list(si.on_wait) if (si is not None and si.on_wait) else []
            if len(waits) > maxw:
                rest = waits[:-maxw]
                si.on_wait = waits[-maxw:]
                for i in range(0, len(rest), maxw):
                    out.append(mybir.InstNoOp(
                        name=f"{inst.name}-wsplit{i}",
                        sync_info=mybir.SyncInfo(on_wait=rest[i:i + maxw], on_update=[]),
                        bass_nofuse=True,
                        engine=inst.engine,
                    ))
            out.append(inst)
        bb.instructions = out


def _dedup_ldw(nc):
    """Drop PE InstLdweights that reload the stationary already resident
    (identical weights AP, no sync side effects). The PE array keeps the
    stationary across matmuls, so consecutive same-weight loads are no-ops."""
    for bb in nc.main_func.blocks:
        out = []
        prev = None
        for inst in bb.instructions:
            if type(inst).__name__ == 'InstLdweights':
                sig = str(inst.ins[0])
                si = inst.sync_info
                clean = not (si and (si.on_wait or si.on_update))
                if sig == prev and clean:
                    continue
                prev = sig
            out.append(inst)
        bb.instructions = out


def _build_nc(reps=1, use_cc=True, mode='full'):
    f32 = mybir.dt.float32
    f32r = mybir.dt.float32r  # noqa: F841
    bf16 = mybir.dt.bfloat16
    i16 = mybir.dt.int16

    nc = bass.Bass(num_swdge_queues=4)
    d_tab = nc.declare_dram_parameter("tab", [128, V], f32, isOutput=False)
    d_idx = nc.declare_dram_parameter("idx", [128, NTOK // 16], i16,
                                      isOutput=False)
    d_stf = nc.declare_dram_parameter("stf", [NC_NOTES, 1], f32, isOutput=False)
    d_w01 = nc.declare_dram_parameter("w01", [128, H], bf16, isOutput=False)
    d_w2 = nc.declare_dram_parameter("w2", [64, H], bf16, isOutput=False)
    d_cb = nc.declare_dram_parameter("convb2", [128, 2], f32, isOutput=False)
    d_iota = nc.declare_dram_parameter("iota", [128, S], f32, isOutput=False)
    d_wz = nc.declare_dram_parameter("wz", [128, 2], f32, isOutput=False)
    d_dw0 = nc.declare_dram_parameter("dw0", [1, NC_NOTES], f32, isOutput=False)
    d_brep = nc.declare_dram_parameter("brep", [128, 1], f32, isOutput=False)
    d_out = nc.declare_dram_parameter("out", [128, 1], f32, isOutput=True)
    part = nc.dram_tensor("part", [S, 2], f32)
    rs_out = nc.dram_tensor("rs_out", [128, 2], f32)

    with tile.TileContext(nc) as tc:
        nc.gpsimd.load_library(library_config.ap_gather)
        with (
            tc.tile_pool(name="cst", bufs=1) as cp,
            tc.tile_pool(name="feat", bufs=1) as fp,
        ):
         for _rep in range(reps):
             w01_sb = cp.tile([128, H], bf16, name="w01sb")
             w2_sb = cp.tile([64, H], bf16, name="w2sb")
             cb_sb = cp.tile([128, 2], f32, name="cbsb")
             nc.sync.dma_start(out=w01_sb[:], in_=d_w01[:])
             nc.sync.dma_start(out=w2_sb[:], in_=d_w2[:])
             nc.sync.dma_start(out=cb_sb[:], in_=d_cb[:])
             feats = [fp.tile([128, NC_NOTES], f32, name=f"feats{hh}") for hh in range(2)]

             # ---- P1: SBUF-resident table, ap_gather -> conv -> maxpool ----
             with tc.tile_pool(name="tabp", bufs=1) as tp:
                 tab = tp.tile([128, V], f32, name="tab")
                 nc.sync.dma_start(out=tab[:], in_=d_tab[:])
                 idx_all = tp.tile([128, NTOK // 16], i16, name="idxall")
                 nc.sync.dma_start(out=idx_all[:], in_=d_idx[:])
                 with (
                     tc.tile_pool(name="gath", bufs=1) as gp,
                     tc.tile_pool(name="ypsum", bufs=1, space="PSUM") as yp,
                 ):
                     for b in range(NBLK):
                         sbf = gp.tile([128, BLK_TOK], bf16, tag="sbf")
                         for hv in range(2):
                             slab = gp.tile([128, BLK_TOK // 2], f32, tag="slab")
                             g = b * NGC + hv
                             nc.gpsimd.ap_gather(
                                 out_ap=slab[:].rearrange("p (n d) -> p n d", d=1),
                                 in_ap=tab[:].rearrange("p (n d) -> p n d", d=1),
                                 idxs_ap=idx_all[:, g * (GCHUNK // 16):
                                                 (g + 1) * (GCHUNK // 16)],
                                 channels=128,
                                 num_elems=V,
                                 d=1,
                                 num_idxs=GCHUNK,
                             )
                             nc.scalar.activation(
                                 out=sbf[:, hv * 4096:(hv + 1) * 4096],
                                 in_=slab[:],
                                 func=mybir.ActivationFunctionType.Copy)
                         if mode == 'gather':
                             continue
                         for hh in range(2):
                             for half in range(2):
                                 c0 = half * 4096
                                 y_ps = yp.tile([128, 4096], f32, tag="y",
                                                name=f"y{_rep}_{b}_{hh}_{half}")
                                 for o in range(0, 4096, 512):
                                     nc.tensor.matmul(
                                         out=y_ps[:, o:o + 512],
                                         lhsT=w01_sb[:, hh * 128:(hh + 1) * 128],
                                         rhs=sbf[:, c0 + o:c0 + o + 512],
                                         start=True, stop=False)
                                 for o in range(0, 4096, 512):
                                     w = 510 if (half == 1 and o == 3584) else 512
                                     nc.tensor.matmul(
                                         out=y_ps[:, o:o + w],
                                         lhsT=w2_sb[:, hh * 128:(hh + 1) * 128],
                                         rhs=sbf[0:64, c0 + o + 2:c0 + o + 2 + w],
                                         start=False, stop=True)
                                 nc.vector.reduce_max(
                                     out=feats[hh][:, b * BLK_NOTES + half * 64:
                                                   b * BLK_NOTES + half * 64 + 64],
                                     in_=y_ps[:].rearrange("p (n l) -> p n l", l=L)
                                         [:, :, 0:TMAX],
                                     axis=mybir.AxisListType.X)

             # ---- P2: relu(feats + conv_b) ----
             if mode == 'gather':
                 nc.vector.memset(feats[0][:], 0.0)
                 nc.vector.memset(feats[1][:], 0.0)
             for hh in range(2):
                 nc.scalar.activation(out=feats[hh][:], in_=feats[hh][:],
                                      func=mybir.ActivationFunctionType.Relu,
                                      bias=cb_sb[:, hh:hh + 1], scale=1.0)

             # ---- P3: per-note scalar z = feats . W[1:257] (+ W0*delta) ----
             with tc.tile_pool(name="zs", bufs=1) as zs:
                 wz_sb = zs.tile([128, 2], f32)
                 nc.sync.dma_start(out=wz_sb[:], in_=d_wz[:])
                 dw0_sb = zs.tile([1, NC_NOTES], f32)
                 nc.sync.dma_start(out=dw0_sb[:], in_=d_dw0[:])
                 z_sb = zs.tile([1, NC_NOTES], f32)
                 with tc.tile_pool(name="zp", bufs=2, space="PSUM") as zp:
                     for q in range(NC_NOTES // 512):
                         z_ps = zp.tile([1, 512], f32, tag="z", name=f"z{_rep}_{q}")
                         for hh in range(2):
                             nc.tensor.matmul(out=z_ps[:],
                                              lhsT=wz_sb[:, hh:hh + 1],
                                              rhs=feats[hh][:, q * 512:(q + 1) * 512],
                                              start=(hh == 0), stop=(hh == 1))
                         nc.vector.tensor_copy(out=z_sb[:, q * 512:(q + 1) * 512],
                                               in_=z_ps[:])
                 nc.vector.tensor_add(out=z_sb[:], in0=z_sb[:], in1=dw0_sb[:])

                 ones11 = zs.tile([1, 1], f32)
                 nc.vector.memset(ones11[:], 1.0)
                 rhs16 = zs.tile([128, 32], f32)
                 with tc.tile_pool(name="tpp", bufs=2, space="PSUM") as tpp:
                     for i in range(NCHUNK):
                         zt_ps = tpp.tile([128, 1], f32, tag="zt", name=f"zt{_rep}_{i}")
                         nc.tensor.matmul(out=zt_ps[:],
                                          lhsT=z_sb[0:1, i * 128:(i + 1) * 128],
                                          rhs=ones11[:],
                                          start=True, stop=True)
                         nc.vector.tensor_copy(out=rhs16[:, 2 * i:2 * i + 1],
                                               in_=zt_ps[:])
                     nc.vector.memset(rhs16[:, 1:32:2], 1.0)

                 # ---- P4: segment-sum of [z, 1] via one-hot matmuls ----
                 with (
                     tc.tile_pool(name="segsb", bufs=2) as ssp,
                     tc.tile_pool(name="segps", bufs=1, space="PSUM") as pp,
                 ):
                     iota_sb = zs.tile([128, S], f32)
                     nc.sync.dma_start(out=iota_sb[:], in_=d_iota[:])
                     st_all = zs.tile([128, NCHUNK], f32)
                     nc.sync.dma_start(
                         out=st_all[:],
                         in_=d_stf[:].rearrange("(i p) o -> p (i o)", p=128))
                     seg_ps = [pp.tile([128, 2], f32, tag=f"seg{bk}",
                                       name=f"seg{_rep}_{bk}")
                               for bk in range(8)]
                     for i in range(NCHUNK):
                         oh_sb = ssp.tile([128, S], f32, tag="oh")
                         nc.vector.tensor_tensor(out=oh_sb[:],
                                                 in0=st_all[:, i:i + 1].to_broadcast([128, S]),
                                                 in1=iota_sb[:],
                                                 op=mybir.AluOpType.is_equal)
                         for bk in range(8):
                             nc.tensor.matmul(out=seg_ps[bk][:],
                                              lhsT=oh_sb[:, bk * 128:(bk + 1) * 128],
                                              rhs=rhs16[:, 2 * i:2 * i + 2],
                                              start=(i == 0), stop=(i == NCHUNK - 1))
                     for bk in range(8):
                         seg_sb = ssp.tile([128, 2], f32, tag="segout")
                         nc.vector.tensor_copy(out=seg_sb[:], in_=seg_ps[bk][:])
                         nc.sync.dma_start(out=part[bk * 128:(bk + 1) * 128, :], in_=seg_sb[:])

             # ---- P5: cross-core reduce + finalize ----
             if use_cc:
                 with tc.tile_critical():
                     with nc.semaphore("cc_sem") as cc_sem:
                         nc.gpsimd.collective_compute(
                             "ReduceScatter", mybir.AluOpType.add,
                             replica_groups=[list(range(NCORES))],
                             ins=[part[:]], outs=[rs_out[:]],
                         ).then_inc(cc_sem, 1)
                         nc.gpsimd.wait_ge(cc_sem, 1)
             else:
                 nc.sync.dma_start(out=rs_out[:], in_=part[0:128, :])

             with tc.tile_pool(name="fin", bufs=1) as fin:
                 brep_sb = fin.tile([128, 1], f32)
                 nc.sync.dma_start(out=brep_sb[:], in_=d_brep[:])
                 fs = fin.tile([128, 2], f32)
                 nc.sync.dma_start(out=fs[:], in_=rs_out[:])
                 cnt = fin.tile([128, 1], f32)
                 nc.vector.tensor_scalar_max(out=cnt[:], in0=fs[:, 1:2], scalar1=1.0)
                 rcp = fin.tile([128, 1], f32)
                 nc.vector.reciprocal(out=rcp[:], in_=cnt[:])
                 dot = fin.tile([128, 1], f32)
                 nc.vector.tensor_tensor(out=dot[:], in0=fs[:, 0:1], in1=rcp[:],
                                         op=mybir.AluOpType.mult)
                 nc.vector.tensor_add(out=dot[:], in0=dot[:], in1=brep_sb[:])
                 outsb = fin.tile([128, 1], f32)
                 nc.scalar.activation(out=outsb[:], in_=dot[:],
                                      func=mybir.ActivationFunctionType.Sigmoid,
                                      scale=1.0)
                 nc.sync.dma_start(out=d_out[:], in_=outsb[:])

    _split_waits(nc)
    mybir.codegen_inst_isa_subclasses(nc)
    _dedup_ldw(nc)
    return nc


_NC_CACHE = {}


def _get_nc(reps=1, use_cc=True, mode='full'):
    key = (reps, use_cc, mode)
    if key not in _NC_CACHE:
        _NC_CACHE[key] = _build_nc(reps, use_cc, mode)
    return _NC_CACHE[key]


def _prep_inputs(text, start_times, emb, conv_w, conv_b, W, b):
    text = np.asarray(text)[0]              # [N, L]
    st = np.asarray(start_times)[0].astype(np.int64)   # [N]
    emb = np.asarray(emb, dtype=np.float32)
    conv_w = np.asarray(conv_w, dtype=np.float32)
    conv_b = np.asarray(conv_b, dtype=np.float32)
    W = np.asarray(W, dtype=np.float32)
    b = np.asarray(b, dtype=np.float32)

    # SBUF-resident transposed table: partitions 0:64 and 64:128 both emb.T
    tab = np.concatenate([emb.T, emb.T], axis=0).astype(np.float32)  # [128, V]
    tab = np.ascontiguousarray(tab)

    import ml_dtypes
    w01 = np.zeros((128, H), dtype=ml_dtypes.bfloat16)
    w01[:64, :] = conv_w[:, :, 0].T.astype(ml_dtypes.bfloat16)
    w01[64:, :] = conv_w[:, :, 1].T.astype(ml_dtypes.bfloat16)
    w2 = np.ascontiguousarray(conv_w[:, :, 2].T.astype(ml_dtypes.bfloat16))
    convb2 = np.ascontiguousarray(conv_b.reshape(2, 128).T.astype(np.float32))

    iota = np.tile(np.arange(S, dtype=np.float32), (128, 1))
    wz = np.ascontiguousarray(W[1:H + 1, 0].reshape(2, 128).T.astype(np.float32))
    brep = np.full((128, 1), b[0], np.float32)

    delta_g = np.concatenate([[0.0], np.diff(st).astype(np.float32)]).astype(np.float32)

    tok = text.astype(np.int16)             # V=30000 < 2**15
    in_maps = []
    for cid in range(NCORES):
        sl = slice(cid * NC_NOTES, (cid + 1) * NC_NOTES)
        flat = tok[sl].reshape(-1)                      # [NTOK] note-major
        # shifted-by-one stream for the k=1 conv tap (clamp last)
        flat_hi = np.concatenate([flat[1:], flat[:1]])
        ngc_all = NTOK // GCHUNK
        idx = np.zeros((128, NTOK // 16), np.int16)
        for g in range(ngc_all):
            lo = flat[g * GCHUNK:(g + 1) * GCHUNK]
            hi = flat_hi[g * GCHUNK:(g + 1) * GCHUNK]
            w_lo = lo.reshape(GCHUNK // 16, 16).T       # [16, GCHUNK//16]
            w_hi = hi.reshape(GCHUNK // 16, 16).T
            cs = slice(g * (GCHUNK // 16), (g + 1) * (GCHUNK // 16))
            idx[0:64, cs] = np.tile(w_lo, (4, 1))
            idx[64:128, cs] = np.tile(w_hi, (4, 1))
        in_maps.append({
            "tab": tab,
            "idx": idx,
            "stf": np.ascontiguousarray(st[sl, None].astype(np.float32)),
            "w01": w01,
            "w2": w2,
            "convb2": convb2,
            "iota": iota,
            "wz": wz,
            "dw0": np.ascontiguousarray((W[0, 0] * delta_g[sl])[None, :]),
            "brep": brep,
        })
    return in_maps


def kernel(**inputs) -> np.ndarray:
    nc = _get_nc()
    in_maps = _prep_inputs(**inputs)
    res = run_bass_kernel_spmd(nc, in_maps, list(range(NCORES))).results
    out = np.concatenate([res[c]["out"] for c in range(NCORES)], axis=0)
    return out.astype(np.float32)


if __name__ == "__main__":
    import jax
    import reference
    cpu = jax.devices("cpu")[0]
    with jax.default_device(cpu):
        ins = {k: np.asarray(v) for k, v in reference.setup_inputs().items()}
        exp = np.asarray(reference.reference(**reference.setup_inputs()))
    got = kernel(**ins)
    err = np.abs(got - exp).max()
    rel = err / max(np.abs(exp).max(), 1e-9)
    print("max abs err:", err, "rel:", rel)
